# revision 1
# baseline (speedup 1.0000x reference)
"""Depthwise cross-correlation (SiamFC-style) Trainium2 kernel.

Problem: template [64,512,8,8] x search [64,512,32,32] -> out [64,512,25,25]
(valid correlation, each (b,c) pair independent).

Batch-parallel across 8 NeuronCores (4096 (b,c) pairs per core). Within
a core, three concurrent lanes split the pairs:

1. PE lane (py): per pair, 8 accumulating 32x32-subarray
   matmuls of host-built banded fp16 template weights against the raw
   search tile, PSUM-accumulated with full-window writes; ScalarE/
   VectorE evacuate PSUM.
2. DVE lane: 64-tap per-partition-scalar fused MAC
   (scalar_tensor_tensor) over shifted free-dim views, fp32.
3. GPSIMD lane: same 64 taps, products computed by ScalarE activation
   (scale=per-partition template tap), accumulated by GPSIMD
   tensor_tensor adds, fp32.
"""

import os
import sys
from contextlib import ExitStack

for _p in ("/opt/trn_rl_repo",):
    if _p not in sys.path:
        sys.path.insert(0, _p)

import numpy as np

import concourse.bacc as bacc
import concourse.mybir as mybir
from concourse.tile import TileContext
from concourse.bass_utils import run_bass_kernel_spmd

# ---- PE lane (TensorEngine) ----

HT, WT = 8, 8
HS, WS = 32, 32
HO, WO = HS - HT + 1, WS - WT + 1  # 25
SLOP = WO  # 25: psum window width (valid columns only)
SPADW = WS  # 32: raw search width per h-copy
NQQ = 2  # matmul passes per pair
NH = 4  # h-shifts folded into K
RPAIRS = 16  # pairs per round (4 psum slots x 4 col positions)
OBATCH = 32  # rounds of output batched into one SBUF tile / DMA set
IBATCH = 4  # rounds of w/s input batched into one DMA
PSBUFS = 6

WCOLS = RPAIRS * NQQ * HO  # 800 weight cols per round
SCOLS = RPAIRS * SPADW  # 512 search cols per round


def host_prep_pe(t_np, s_np):
    """t_np [N,8,8] f32, s_np [N,1024] f32 (N multiple of 16) ->
    (w [R/IB,128,IB*800] fp16, s [R/IB,128,IB*512] fp16), R = N/16."""
    n = t_np.shape[0]
    r = n // RPAIRS
    tt = t_np.astype(np.float16)

    # w[pair, qq, h, rr, i] = T[pair, rr-i, 4qq+h]  (0 <= rr-i < 8)
    w = np.zeros((n, NQQ, NH, HS, HO), np.float16)
    idx = np.arange(HO)
    for p in range(HT):
        for q in range(WT):
            qq, h = divmod(q, NH)
            w[:, qq, h, idx + p, idx] = tt[:, p, q, None]
    # -> [R, pair16, qq, (h rr)128, i25] -> [R, (h rr)128, (pair qq i)800]
    w = w.reshape(r, RPAIRS, NQQ, NH * HS, HO)
    w = w.transpose(0, 3, 1, 2, 4).reshape(r, NH * HS, WCOLS)
    w = w.reshape(r // IBATCH, IBATCH, 128, WCOLS)
    w = w.transpose(0, 2, 1, 3).reshape(r // IBATCH, 128, IBATCH * WCOLS)

    sp = np.zeros((n, HS, WS + NH), np.float16)
    sp[:, :, 0:WS] = s_np.astype(np.float16).reshape(n, HS, WS)
    # s4[pair, h, rr, x] = S[pair, rr, x+h], x in [0,32)
    s4 = np.stack([sp[:, :, h : h + SPADW] for h in range(NH)], axis=1)
    # -> [R, pair16, (h rr)128, x32] -> [R, (h rr)128, (pair x)512]
    s4 = s4.reshape(r, RPAIRS, NH * HS, SPADW)
    s4 = s4.transpose(0, 2, 1, 3).reshape(r, NH * HS, SCOLS)
    s4 = s4.reshape(r // IBATCH, IBATCH, 128, SCOLS)
    s4 = s4.transpose(0, 2, 1, 3).reshape(r // IBATCH, 128, IBATCH * SCOLS)
    return np.ascontiguousarray(w), np.ascontiguousarray(s4)


def build_pe(ctx: ExitStack, nc, tc, w, s, o, rounds, pools=None, evac_engines="both"):
    """Emit the PE pipeline for `rounds` rounds.

    w:  DRAM [rounds/IBATCH, 128, IBATCH*800] fp16 (banded weights)
    s:  DRAM [rounds/IBATCH, 128, IBATCH*512] fp16 (h-shifted search)
    o:  DRAM [rounds*16, 625] fp16 view (output pairs for this lane)
    """
    f16 = mybir.dt.float16
    f32 = mybir.dt.float32
    if pools is None:
        wpool = ctx.enter_context(tc.tile_pool(name="pe_w", bufs=4))
        spool = ctx.enter_context(tc.tile_pool(name="pe_s", bufs=4))
        ppool = ctx.enter_context(tc.tile_pool(name="pe_ps", bufs=PSBUFS, space="PSUM"))
        opool = ctx.enter_context(tc.tile_pool(name="pe_o", bufs=2))
        zpool = ctx.enter_context(tc.tile_pool(name="pe_z", bufs=1))
    else:
        wpool, spool, ppool, opool, zpool = pools

    # Zero-fill every psum pool slot once so the partitions M=25 never
    # writes read as finite zeros in the batched evacuation copies.
    z_t = zpool.tile([128, 512], f16, tag="pe_z")
    nc.vector.memset(z_t[:], 0.0)
    pre = []
    for _ in range(PSBUFS):
        pz = ppool.tile([128, 512], f32, tag="pe_ps")
        nc.tensor.matmul(
            pz[:, :], z_t[:, 0:128], z_t[:, :], start=True, stop=True,
            skip_group_check=True,
        )
        pre.append(pz)
    del pre

    assert rounds % OBATCH == 0 and rounds % IBATCH == 0
    for rr in range(rounds):
        ib = rr % IBATCH
        if ib == 0:
            ws_t = wpool.tile([128, IBATCH * WCOLS], f16, tag="pe_w")
            nc.sync.dma_start(out=ws_t[:], in_=w[rr // IBATCH])
            ss_t = spool.tile([128, IBATCH * SCOLS], f16, tag="pe_s")
            nc.sync.dma_start(out=ss_t[:], in_=s[rr // IBATCH])
        w_t = ws_t[:, ib * WCOLS : (ib + 1) * WCOLS]
        s_t = ss_t[:, ib * SCOLS : (ib + 1) * SCOLS]

        if rr % OBATCH == 0:
            o_t = opool.tile([128, OBATCH * 4, WO], f16, tag="pe_o")
        ps = ppool.tile([128, 512], f32, tag="pe_ps")
        for pp in range(RPAIRS):
            slot, c = divmod(pp, 4)
            for qq in range(NQQ):
                lhs = w_t[:, (pp * NQQ + qq) * HO : (pp * NQQ + qq + 1) * HO]
                rhs = s_t[:, pp * SPADW + NH * qq : pp * SPADW + NH * qq + SLOP]
                nc.tensor.matmul(
                    ps[32 * c : 32 * c + HO, slot * 128 : slot * 128 + SLOP],
                    lhs,
                    rhs,
                    start=(qq == 0),
                    stop=(qq == NQQ - 1),
                    tile_position=(0, 32 * c),
                    skip_group_check=True,
                )
        ps4 = ps[:].rearrange("p (g x) -> p g x", g=4)
        dst_slot = o_t[:, (rr % OBATCH) * 4 : (rr % OBATCH) * 4 + 4, :]
        use_dve = evac_engines != "act" and rr % 2 == 1
        if use_dve:
            nc.vector.tensor_copy(dst_slot, ps4[:, :, 0:WO])
        else:
            nc.scalar.copy(dst_slot, ps4[:, :, 0:WO])

        if rr % OBATCH == OBATCH - 1:
            # DRAM pair row = base + slot*4 + c ; columns i*25+j.
            # One DMA per c: src [25p, OBATCH*4, 25], dst rows stepped by 4.
            base = (rr - (OBATCH - 1)) * RPAIRS
            for c in range(4):
                dst = o[base + c : base + OBATCH * RPAIRS : 4, :].rearrange(
                    "rg (i j) -> i rg j", j=WO
                )
                src = o_t[32 * c : 32 * c + HO, :, :]
                nc.sync.dma_start(out=dst, in_=src)

# ---- end PE lane ----


B, C = 64, 512
HT, WT = 8, 8
HS, WS = 32, 32
HO, WO = HS - HT + 1, WS - WT + 1  # 25, 25
NCORES = 8
PAIRS = (B // NCORES) * C  # 4096 (b,c) pairs per core
GROUP = 128

PE_PAIRS = int(os.environ.get("K_PE_PAIRS", "4096"))  # mult of 16*OBATCH
DVE_PAIRS = int(os.environ.get("K_DVE_PAIRS", "0"))  # multiple of 128
GP_PAIRS = PAIRS - PE_PAIRS - DVE_PAIRS  # multiple of 128
assert PE_PAIRS % (RPAIRS * OBATCH) == 0
assert DVE_PAIRS % GROUP == 0 and GP_PAIRS % GROUP == 0 and GP_PAIRS >= 0

_CACHED_NC = None


def _vector_lane(nc, tc, ctx, t, s, o, n_pairs, engine):
    """64-tap shift-MAC over `n_pairs` pairs (groups of 128).

    engine == 'dve': fused scalar_tensor_tensor on VectorE.
    engine == 'gp':  products on ScalarE (activation scale), adds on GpSimd.
    """
    f32 = mybir.dt.float32
    mult = mybir.AluOpType.mult
    add = mybir.AluOpType.add
    copy_f = mybir.ActivationFunctionType.Copy

    spool = ctx.enter_context(tc.tile_pool(name=f"{engine}_s", bufs=2))
    tpool = ctx.enter_context(tc.tile_pool(name=f"{engine}_t", bufs=2))
    apool = ctx.enter_context(tc.tile_pool(name=f"{engine}_a", bufs=2))
    mpool = (
        ctx.enter_context(tc.tile_pool(name=f"{engine}_m", bufs=3))
        if engine == "gp"
        else None
    )

    for g in range(n_pairs // GROUP):
        row = slice(g * GROUP, (g + 1) * GROUP)
        s_tile = spool.tile([GROUP, HS, WS], f32, tag=f"{engine}_s")
        nc.sync.dma_start(
            out=s_tile[:], in_=s[row, :].rearrange("p (h w) -> p h w", h=HS)
        )
        t_tile = tpool.tile([GROUP, HT * WT], f32, tag=f"{engine}_t")
        nc.sync.dma_start(out=t_tile[:], in_=t[row, :])

        acc = apool.tile([GROUP, HO, WO], f32, tag=f"{engine}_a")
        for k in range(HT * WT):
            p, q = divmod(k, WT)
            win = s_tile[:, p : p + HO, q : q + WO]
            tk = t_tile[:, k : k + 1]
            if engine == "dve":
                if k == 0:
                    nc.vector.tensor_scalar_mul(acc[:], win, tk)
                else:
                    nc.vector.scalar_tensor_tensor(acc[:], win, tk, acc[:], mult, add)
            else:
                if k == 0:
                    nc.scalar.activation(acc[:], win, copy_f, scale=tk)
                else:
                    tmp = mpool.tile([GROUP, HO, WO], f32, tag="gp_tmp")
                    nc.scalar.activation(tmp[:], win, copy_f, scale=tk)
                    nc.gpsimd.tensor_tensor(acc[:], tmp[:], acc[:], add)

        nc.sync.dma_start(
            out=o[row, :].rearrange("p (h w) -> p h w", h=HO), in_=acc[:]
        )


def _build_program():
    global _CACHED_NC
    if _CACHED_NC is not None:
        return _CACHED_NC

    nc = bacc.Bacc()
    f32 = mybir.dt.float32
    f16 = mybir.dt.float16

    pe_rounds = PE_PAIRS // RPAIRS
    nv = PAIRS - PE_PAIRS
    if pe_rounds:
        o16 = nc.declare_dram_parameter(
            "o16", [PE_PAIRS, HO * WO], f16, isOutput=True
        )
        ib = IBATCH
        w = nc.declare_dram_parameter(
            "w", [pe_rounds // ib, 128, ib * WCOLS], f16, isOutput=False
        )
        sp = nc.declare_dram_parameter(
            "sp", [pe_rounds // ib, 128, ib * SCOLS], f16, isOutput=False
        )
    if nv:
        o = nc.declare_dram_parameter("o", [nv, HO * WO], f32, isOutput=True)
        t = nc.declare_dram_parameter("t", [nv, HT * WT], f32, isOutput=False)
        s = nc.declare_dram_parameter("s", [nv, HS * WS], f32, isOutput=False)

    with TileContext(nc) as tc:
        with ExitStack() as ctx:
            if pe_rounds:
                build_pe(
                    ctx, nc, tc, w, sp, o16, pe_rounds,
                    evac_engines="act" if nv else "both",
                )
            if DVE_PAIRS:
                _vector_lane(
                    nc, tc, ctx,
                    t[0:DVE_PAIRS, :], s[0:DVE_PAIRS, :],
                    o[0:DVE_PAIRS, :],
                    DVE_PAIRS, "dve",
                )
            if GP_PAIRS:
                _vector_lane(
                    nc, tc, ctx,
                    t[DVE_PAIRS:nv, :], s[DVE_PAIRS:nv, :],
                    o[DVE_PAIRS:nv, :],
                    GP_PAIRS, "gp",
                )

    nc.finalize()
    _CACHED_NC = nc
    return nc


def kernel(template_features, search_features):
    nc = _build_program()
    tf = np.ascontiguousarray(template_features, dtype=np.float32).reshape(
        NCORES, PAIRS, HT * WT
    )
    sf = np.ascontiguousarray(search_features, dtype=np.float32).reshape(
        NCORES, PAIRS, HS * WS
    )
    in_maps = []
    for i in range(NCORES):
        m = {}
        if PE_PAIRS:
            w_h, sp_h = host_prep_pe(
                tf[i, :PE_PAIRS].reshape(-1, HT, WT), sf[i, :PE_PAIRS]
            )
            m["w"], m["sp"] = w_h, sp_h
        if PAIRS - PE_PAIRS:
            m["t"] = tf[i, PE_PAIRS:]
            m["s"] = sf[i, PE_PAIRS:]
        in_maps.append(m)

    res = run_bass_kernel_spmd(nc, in_maps, list(range(NCORES)))
    global LAST_RESULT
    LAST_RESULT = res
    parts = []
    for r in res.results:
        rows = []
        if PE_PAIRS:
            rows.append(r["o16"].astype(np.float32))
        if PAIRS - PE_PAIRS:
            rows.append(r["o"])
        parts.append(np.concatenate(rows, axis=0))
    return np.stack(parts).reshape(B, C, HO, WO).astype(np.float32)


LAST_RESULT = None


if __name__ == "__main__":
    rng = np.random.default_rng(0)
    tf = rng.standard_normal((B, C, HT, WT), dtype=np.float32)
    sf = rng.standard_normal((B, C, HS, WS), dtype=np.float32)
    out = kernel(tf, sf)
    print("kernel output", out.shape, out.dtype, float(np.abs(out).mean()))



# revision 8
# speedup vs baseline: 1.9509x; 1.9509x over previous
"""Depthwise cross-correlation (SiamFC-style) Trainium2 kernel.

Problem: template [64,512,8,8] x search [64,512,32,32] -> out [64,512,25,25]
(valid correlation, each (b,c) pair independent).

Batch-parallel across 8 NeuronCores (4096 (b,c) pairs per core). Within
a core, three concurrent lanes split the pairs:

1. PE lane (py): per pair, 8 accumulating 32x32-subarray
   matmuls of host-built banded fp16 template weights against the raw
   search tile, PSUM-accumulated with full-window writes; ScalarE/
   VectorE evacuate PSUM.
2. DVE lane: 64-tap per-partition-scalar fused MAC
   (scalar_tensor_tensor) over shifted free-dim views, fp32.
3. GPSIMD lane: same 64 taps, products computed by ScalarE activation
   (scale=per-partition template tap), accumulated by GPSIMD
   tensor_tensor adds, fp32.
"""

import os
import sys
from contextlib import ExitStack

for _p in ("/opt/trn_rl_repo",):
    if _p not in sys.path:
        sys.path.insert(0, _p)

import ml_dtypes
import numpy as np

import concourse.bacc as bacc
import concourse.mybir as mybir
from concourse.tile import TileContext
from concourse.bass_utils import run_bass_kernel_spmd

# ---- PE lane (TensorEngine) ----

HT, WT = 8, 8
HS, WS = 32, 32
HO, WO = HS - HT + 1, WS - WT + 1  # 25
SLOP = WO  # 25: psum window width (valid columns only)
SPADW = WS  # 32: raw search width per h-copy
NQQ = 2  # matmul passes per pair
NH = 4  # h-shifts folded into K
RPAIRS = 16  # pairs per round (4 psum slots x 4 col positions)
OBATCH = 32  # rounds of output batched into one SBUF tile / DMA set
IBATCH = 4  # rounds of w/s input batched into one DMA
PSBUFS = 6

WCOLS = RPAIRS * NQQ * HO  # 800 weight cols per round
SCOLS = RPAIRS * SPADW  # 512 search cols per round


F8 = ml_dtypes.float8_e3m4


def host_prep_pe(t_np, s_np):
    """t_np [N,8,8] f32, s_np [N,1024] f32 (N multiple of 16) ->
    (w [R/IB,128,IB*800] fp8e3, s [R/IB,128,IB*512] fp8e3), R = N/16."""
    n = t_np.shape[0]
    r = n // RPAIRS
    tt = t_np.astype(F8)

    # w[pair, qq, h, rr, i] = T[pair, rr-i, 4qq+h]  (0 <= rr-i < 8)
    w = np.zeros((n, NQQ, NH, HS, HO), F8)
    idx = np.arange(HO)
    for p in range(HT):
        for q in range(WT):
            qq, h = divmod(q, NH)
            w[:, qq, h, idx + p, idx] = tt[:, p, q, None]
    # -> [R, pair16, qq, (h rr)128, i25] -> [R, (h rr)128, (pair qq i)800]
    w = w.reshape(r, RPAIRS, NQQ, NH * HS, HO)
    w = w.transpose(0, 3, 1, 2, 4).reshape(r, NH * HS, WCOLS)
    w = w.reshape(r // IBATCH, IBATCH, 128, WCOLS)
    w = w.transpose(0, 2, 1, 3).reshape(r // IBATCH, 128, IBATCH * WCOLS)

    sp = np.zeros((n, HS, WS + NH), F8)
    sp[:, :, 0:WS] = s_np.astype(F8).reshape(n, HS, WS)
    # s4[pair, h, rr, x] = S[pair, rr, x+h], x in [0,32)
    s4 = np.stack([sp[:, :, h : h + SPADW] for h in range(NH)], axis=1)
    # -> [R, pair16, (h rr)128, x32] -> [R, (h rr)128, (pair x)512]
    s4 = s4.reshape(r, RPAIRS, NH * HS, SPADW)
    s4 = s4.transpose(0, 2, 1, 3).reshape(r, NH * HS, SCOLS)
    s4 = s4.reshape(r // IBATCH, IBATCH, 128, SCOLS)
    s4 = s4.transpose(0, 2, 1, 3).reshape(r // IBATCH, 128, IBATCH * SCOLS)
    return np.ascontiguousarray(w), np.ascontiguousarray(s4)


def build_pe(ctx: ExitStack, nc, tc, w, s, o, rounds, pools=None, evac_engines="both"):
    """Emit the PE pipeline for `rounds` rounds.

    w:  DRAM [rounds/IBATCH, 128, IBATCH*800] fp8e3 (banded weights)
    s:  DRAM [rounds/IBATCH, 128, IBATCH*512] fp8e3 (h-shifted search)
    o:  DRAM [rounds/OBATCH, 4, 25, OBATCH*4, 25] fp16 (blocked output;
        host unscrambles to pair rows)
    """
    f8 = mybir.dt.float8e3
    f16 = mybir.dt.float16
    f32 = mybir.dt.float32
    if pools is None:
        wpool = ctx.enter_context(tc.tile_pool(name="pe_w", bufs=4))
        spool = ctx.enter_context(tc.tile_pool(name="pe_s", bufs=4))
        ppool = ctx.enter_context(tc.tile_pool(name="pe_ps", bufs=PSBUFS, space="PSUM"))
        opool = ctx.enter_context(tc.tile_pool(name="pe_o", bufs=2))
        zpool = ctx.enter_context(tc.tile_pool(name="pe_z", bufs=1))
    else:
        wpool, spool, ppool, opool, zpool = pools

    # Zero-fill every psum pool slot once so the partitions M=25 never
    # writes read as finite zeros in the batched evacuation copies.
    z_t = zpool.tile([128, 512], f16, tag="pe_z")
    nc.vector.memset(z_t[:], 0.0)
    pre = []
    for _ in range(PSBUFS):
        pz = ppool.tile([128, 512], f32, tag="pe_ps")
        nc.tensor.matmul(
            pz[:, :], z_t[:, 0:128], z_t[:, :], start=True, stop=True,
            skip_group_check=True,
        )
        pre.append(pz)
    del pre

    assert rounds % OBATCH == 0 and rounds % IBATCH == 0
    for rr in range(rounds):
        ib = rr % IBATCH
        if ib == 0:
            ws_t = wpool.tile([128, IBATCH * WCOLS], f8, tag="pe_w")
            nc.sync.dma_start(out=ws_t[:], in_=w[rr // IBATCH])
            ss_t = spool.tile([128, IBATCH * SCOLS], f8, tag="pe_s")
            nc.sync.dma_start(out=ss_t[:], in_=s[rr // IBATCH])
        w_t = ws_t[:, ib * WCOLS : (ib + 1) * WCOLS]
        s_t = ss_t[:, ib * SCOLS : (ib + 1) * SCOLS]

        if rr % OBATCH == 0:
            o_t = opool.tile([128, OBATCH * 4, WO], f16, tag="pe_o")
        ps = ppool.tile([128, 512], f32, tag="pe_ps")
        for pp in range(RPAIRS):
            slot, c = divmod(pp, 4)
            for qq in range(NQQ):
                lhs = w_t[:, (pp * NQQ + qq) * HO : (pp * NQQ + qq + 1) * HO]
                rhs = s_t[:, pp * SPADW + NH * qq : pp * SPADW + NH * qq + SLOP]
                nc.tensor.matmul(
                    ps[32 * c : 32 * c + HO, slot * 128 : slot * 128 + SLOP],
                    lhs,
                    rhs,
                    start=(qq == 0),
                    stop=(qq == NQQ - 1),
                    tile_position=(0, 32 * c),
                    skip_group_check=True,
                )
        ps4 = ps[:].rearrange("p (g x) -> p g x", g=4)
        dst_slot = o_t[:, (rr % OBATCH) * 4 : (rr % OBATCH) * 4 + 4, :]
        use_dve = evac_engines != "act" and rr % 2 == 1
        if use_dve:
            nc.vector.tensor_copy(dst_slot, ps4[:, :, 0:WO])
        else:
            nc.scalar.copy(dst_slot, ps4[:, :, 0:WO])

        if rr % OBATCH == OBATCH - 1:
            # Blocked store: o[ob, c, i, rg, j] <- o_t[32c+i, rg, j].
            # Per-partition free span (OBATCH*4*25 fp16 = 6400B) is
            # contiguous on both sides -> 25 big descriptors per DMA.
            ob = rr // OBATCH
            for c in range(4):
                src = o_t[32 * c : 32 * c + HO, :, :]
                nc.sync.dma_start(out=o[ob, c], in_=src)

# ---- end PE lane ----


B, C = 64, 512
HT, WT = 8, 8
HS, WS = 32, 32
HO, WO = HS - HT + 1, WS - WT + 1  # 25, 25
NCORES = 8
PAIRS = (B // NCORES) * C  # 4096 (b,c) pairs per core
GROUP = 128

PE_PAIRS = int(os.environ.get("K_PE_PAIRS", "4096"))  # mult of 16*OBATCH
DVE_PAIRS = int(os.environ.get("K_DVE_PAIRS", "0"))  # multiple of 128
GP_PAIRS = PAIRS - PE_PAIRS - DVE_PAIRS  # multiple of 128
assert PE_PAIRS % (RPAIRS * OBATCH) == 0
assert DVE_PAIRS % GROUP == 0 and GP_PAIRS % GROUP == 0 and GP_PAIRS >= 0

_CACHED_NC = None


def _vector_lane(nc, tc, ctx, t, s, o, n_pairs, engine):
    """64-tap shift-MAC over `n_pairs` pairs (groups of 128).

    engine == 'dve': fused scalar_tensor_tensor on VectorE.
    engine == 'gp':  products on ScalarE (activation scale), adds on GpSimd.
    """
    f32 = mybir.dt.float32
    mult = mybir.AluOpType.mult
    add = mybir.AluOpType.add
    copy_f = mybir.ActivationFunctionType.Copy

    spool = ctx.enter_context(tc.tile_pool(name=f"{engine}_s", bufs=2))
    tpool = ctx.enter_context(tc.tile_pool(name=f"{engine}_t", bufs=2))
    apool = ctx.enter_context(tc.tile_pool(name=f"{engine}_a", bufs=2))
    mpool = (
        ctx.enter_context(tc.tile_pool(name=f"{engine}_m", bufs=3))
        if engine == "gp"
        else None
    )

    for g in range(n_pairs // GROUP):
        row = slice(g * GROUP, (g + 1) * GROUP)
        s_tile = spool.tile([GROUP, HS, WS], f32, tag=f"{engine}_s")
        nc.sync.dma_start(
            out=s_tile[:], in_=s[row, :].rearrange("p (h w) -> p h w", h=HS)
        )
        t_tile = tpool.tile([GROUP, HT * WT], f32, tag=f"{engine}_t")
        nc.sync.dma_start(out=t_tile[:], in_=t[row, :])

        acc = apool.tile([GROUP, HO, WO], f32, tag=f"{engine}_a")
        for k in range(HT * WT):
            p, q = divmod(k, WT)
            win = s_tile[:, p : p + HO, q : q + WO]
            tk = t_tile[:, k : k + 1]
            if engine == "dve":
                if k == 0:
                    nc.vector.tensor_scalar_mul(acc[:], win, tk)
                else:
                    nc.vector.scalar_tensor_tensor(acc[:], win, tk, acc[:], mult, add)
            else:
                if k == 0:
                    nc.scalar.activation(acc[:], win, copy_f, scale=tk)
                else:
                    tmp = mpool.tile([GROUP, HO, WO], f32, tag="gp_tmp")
                    nc.scalar.activation(tmp[:], win, copy_f, scale=tk)
                    nc.gpsimd.tensor_tensor(acc[:], tmp[:], acc[:], add)

        nc.sync.dma_start(
            out=o[row, :].rearrange("p (h w) -> p h w", h=HO), in_=acc[:]
        )


def _build_program():
    global _CACHED_NC
    if _CACHED_NC is not None:
        return _CACHED_NC

    nc = bacc.Bacc()
    f32 = mybir.dt.float32
    f16 = mybir.dt.float16

    pe_rounds = PE_PAIRS // RPAIRS
    nv = PAIRS - PE_PAIRS
    if pe_rounds:
        o16 = nc.declare_dram_parameter(
            "o16",
            [pe_rounds // OBATCH, 4, HO, OBATCH * 4, WO],
            f16,
            isOutput=True,
        )
        ib = IBATCH
        f8 = mybir.dt.float8e3
        w = nc.declare_dram_parameter(
            "w", [pe_rounds // ib, 128, ib * WCOLS], f8, isOutput=False
        )
        sp = nc.declare_dram_parameter(
            "sp", [pe_rounds // ib, 128, ib * SCOLS], f8, isOutput=False
        )
    if nv:
        o = nc.declare_dram_parameter("o", [nv, HO * WO], f32, isOutput=True)
        t = nc.declare_dram_parameter("t", [nv, HT * WT], f32, isOutput=False)
        s = nc.declare_dram_parameter("s", [nv, HS * WS], f32, isOutput=False)

    with TileContext(nc) as tc:
        with ExitStack() as ctx:
            if pe_rounds:
                build_pe(
                    ctx, nc, tc, w, sp, o16, pe_rounds,
                    evac_engines="act" if nv else "both",
                )
            if DVE_PAIRS:
                _vector_lane(
                    nc, tc, ctx,
                    t[0:DVE_PAIRS, :], s[0:DVE_PAIRS, :],
                    o[0:DVE_PAIRS, :],
                    DVE_PAIRS, "dve",
                )
            if GP_PAIRS:
                _vector_lane(
                    nc, tc, ctx,
                    t[DVE_PAIRS:nv, :], s[DVE_PAIRS:nv, :],
                    o[DVE_PAIRS:nv, :],
                    GP_PAIRS, "gp",
                )

    nc.finalize()
    _CACHED_NC = nc
    return nc


def kernel(template_features, search_features):
    nc = _build_program()
    tf = np.ascontiguousarray(template_features, dtype=np.float32).reshape(
        NCORES, PAIRS, HT * WT
    )
    sf = np.ascontiguousarray(search_features, dtype=np.float32).reshape(
        NCORES, PAIRS, HS * WS
    )
    in_maps = []
    for i in range(NCORES):
        m = {}
        if PE_PAIRS:
            w_h, sp_h = host_prep_pe(
                tf[i, :PE_PAIRS].reshape(-1, HT, WT), sf[i, :PE_PAIRS]
            )
            m["w"], m["sp"] = w_h, sp_h
        if PAIRS - PE_PAIRS:
            m["t"] = tf[i, PE_PAIRS:]
            m["s"] = sf[i, PE_PAIRS:]
        in_maps.append(m)

    res = run_bass_kernel_spmd(nc, in_maps, list(range(NCORES)))
    global LAST_RESULT
    LAST_RESULT = res
    parts = []
    for r in res.results:
        rows = []
        if PE_PAIRS:
            # [ob, c, i, (rb slot), j] -> pair = ((ob*OB + rb)*16 + slot*4 + c
            blk = r["o16"].reshape(-1, 4, HO, OBATCH, 4, WO)
            blk = blk.transpose(0, 3, 4, 1, 2, 5).reshape(PE_PAIRS, HO * WO)
            rows.append(blk.astype(np.float32))
        if PAIRS - PE_PAIRS:
            rows.append(r["o"])
        parts.append(np.concatenate(rows, axis=0))
    return np.stack(parts).reshape(B, C, HO, WO).astype(np.float32)


LAST_RESULT = None


if __name__ == "__main__":
    rng = np.random.default_rng(0)
    tf = rng.standard_normal((B, C, HT, WT), dtype=np.float32)
    sf = rng.standard_normal((B, C, HS, WS), dtype=np.float32)
    out = kernel(tf, sf)
    print("kernel output", out.shape, out.dtype, float(np.abs(out).mean()))

